# revision 33
# baseline (speedup 1.0000x reference)
"""CrossAttention kernel for Trainium2 (8 NeuronCores, SPMD).

Reference math (B=4, C=256, N=4096, OUT=256, TEMP=sqrt(OUT)=16):
    q = Wq @ x          (B, OUT, N)
    k = Wk @ xx         (B, OUT, N)
    v = Wv @ xx         (B, OUT, N)
    attn = softmax(q^T k / TEMP, axis=-1)   (B, N, N)
    y = einsum('bnm,bom->bon', attn, v)     (B, OUT, N)

Sharding: 8 cores = (batch b, query-half h); each core computes its 2048
query rows against the full 4096 keys of its batch.

Per-core kernel:
    q   = Wq @ x   -> (OUT, bc)  bf16 matmul; fp8 hi + fp8 lo residual
    k   = Wk @ xx  -> (OUT, m)   bf16 matmul; fp8
    vT  = xx^T @ Wv^T -> (m, OUT) bf16 matmul; fp8 (+ exact ones column)
    S_T = k^T (q_hi + q_lo)  (fp8 DoubleRow: 256-deep contraction/instr)
    P   = exp(S_T/TEMP) fp32 on ACT;  P' = P - 1 -> fp8 (DVE + GPSIMD)
    dev = P'^T @ v_aug  (fp8 DoubleRow over m)  -> (n, OUT+1) bf16 out
    host: y = ((dev[:, :OUT] + colsum(v)) / (dev[:, OUT] + m)).T

Numerics: logits are ~N(0, 1/9) so exp(S) is centered at 1; quantizing
P' = exp(S)-1 to fp8e4 gives ~3x smaller error than exp(S), and the
query-side hi/lo split removes the dominant S error term (per-row logit
noise; key-side noise mostly cancels via the denominator). The missing
ones-row term 1@v_aug = [colsum(v), m] is added back on the host in
fp64, which also keeps the bf16 device output small (deviations ~±40,
not 4096-ish totals).

Schedule: ACT's exp stream (~67us busy/iteration) is the bottleneck and
runs exp-ONLY; everything else is arranged to keep it fed:
 - all tile pools are program-lifetime (pool open/close emits all-engine
   barriers that would serialize phases and iterations);
 - repeat iterations are software-pipelined at emission level: each
   iteration's input DMAs + q/k projection chain are emitted inside the
   PREVIOUS iteration's block-3 stream (xq/xkv/k8/v8 double-buffered as
   needed), and PV stages interleave one iteration behind their S/exp;
 - elementwise work is spread: DVE does copies + most P-1 subs, GPSIMD
   (otherwise idle, SBUF->SBUF capable) takes a fraction of the subs,
   SP dispatches input DMAs, the Pool queue dispatches output DMAs.
"""

import numpy as np
import ml_dtypes
from contextlib import ExitStack

import concourse.bass as bass
import concourse.tile as tile
from concourse import bacc, mybir
from concourse.bass_utils import run_bass_kernel_spmd

B, C, NSEQ, OUT = 4, 256, 4096, 256
TEMP = float(OUT) ** 0.5
NCORES = 8
BF16 = mybir.dt.bfloat16
F32 = mybir.dt.float32
FP8 = mybir.dt.float8e4
BFNP = ml_dtypes.bfloat16
DR = mybir.MatmulPerfMode.DoubleRow

EXP = mybir.ActivationFunctionType.Exp


def build(bc=2048, m=4096, nblk=512, repeat_full=1, subs_on_pool=5):
    """Build the per-core SPMD Bass program.

    bc: query rows per core; m: key count; nblk: query block width
    (nblk*4B <= one PSUM bank). repeat_full: re-run the whole body R times
    (perf measurement only). subs_on_pool: of every 16 P-1 subs per block,
    how many run on GPSIMD instead of DVE.
    """
    ct = C // 128     # contraction tiles for the 1x1-conv projections
    ot = OUT // 128   # output-channel tiles (2 -> one DoubleRow pair)
    mt = m // 128     # key tiles
    nb = bc // nblk   # query blocks
    nt = nblk // 128  # 128-query tiles per block
    qch = bc // 512
    kch = m // 512

    nc = bacc.Bacc("TRN2", target_bir_lowering=False, debug=False,
                   num_devices=NCORES)
    xq_d = nc.dram_tensor("xq", [ct, 128, bc], BF16, kind="ExternalInput")
    xkv_d = nc.dram_tensor("xkv", [ct, 128, m], BF16, kind="ExternalInput")
    wq_d = nc.dram_tensor("wqT", [ct, 128, OUT], BF16, kind="ExternalInput")
    wk_d = nc.dram_tensor("wkT", [ct, 128, OUT], BF16, kind="ExternalInput")
    wv_d = nc.dram_tensor("wvT", [ct, 128, OUT], BF16, kind="ExternalInput")
    y_d = nc.dram_tensor("y", [bc // 128, 128, OUT + 1], BF16,
                         kind="ExternalOutput")

    with tile.TileContext(nc) as tc, ExitStack() as ctx:
        const = ctx.enter_context(tc.tile_pool(name="const", bufs=1))

        xq_sb = const.tile([128, ct, bc], BF16, name="xq_sb")
        wq_sb = const.tile([128, ct, OUT], BF16, name="wq_sb")
        wk_sb = const.tile([128, ct, OUT], BF16, name="wk_sb")
        wv_sb = const.tile([128, ct, OUT], BF16, name="wv_sb")
        q8h = const.tile([128, ot, bc], FP8, name="q8h")
        q8l = const.tile([128, ot, bc], FP8, name="q8l")
        # xkv/k8/v8 double-buffered: iteration i+1's DMAs + projections use
        # one copy while iteration i's S/PV stages still read the other
        xkvs = [const.tile([128, ct, m], BF16, name=f"xkv_{j}")
                for j in range(2)]
        k8s = [const.tile([128, ot, m], FP8, name=f"k8_{j}") for j in range(2)]
        v8s = [const.tile([128, mt, OUT + 1], FP8, name=f"v8_{j}")
               for j in range(2)]

        # the ones column of v_aug (for the softmax denominator) is static
        # across repeats
        for v8 in v8s:
            nc.vector.memset(v8[:, :, OUT:OUT + 1], 1.0)

        # all pools live for the whole program: pool open/close emits
        # all-engine barriers, which would serialize repeat iterations (and
        # phases within one) instead of letting them pipeline. Static PSUM
        # budget: s 2x[128,2,512] (4 banks) + proj 1x[128,2,512] (2 banks)
        # + y 2x[128,257] (2 banks) = 8.
        p_pool = ctx.enter_context(tc.tile_pool(name="p_sb", bufs=5))
        pf_pool = ctx.enter_context(tc.tile_pool(name="pf_sb", bufs=5))
        ysb_pool = ctx.enter_context(tc.tile_pool(name="y_sb", bufs=4))
        s_pool = ctx.enter_context(
            tc.tile_pool(name="s_ps", bufs=2, space="PSUM"))
        proj_pool = ctx.enter_context(
            tc.tile_pool(name="proj_ps", bufs=1, space="PSUM"))
        y_pool = ctx.enter_context(
            tc.tile_pool(name="y_ps", bufs=2, space="PSUM"))

        def emit_dmas(rf):
            # DMA order = consumption order (each dma_start costs ~0.6us of
            # serialized SP dispatch, so order is latency-critical)
            xkv_sb = xkvs[rf % 2]
            for i in range(ct):
                nc.sync.dma_start(wq_sb[:, i, :], wq_d.ap()[i])
            for i in range(ct):
                nc.sync.dma_start(xq_sb[:, i, 0:512], xq_d.ap()[i][:, 0:512])
            for i in range(ct):
                nc.sync.dma_start(wk_sb[:, i, :], wk_d.ap()[i])
            for i in range(ct):
                nc.sync.dma_start(
                    xkv_sb[:, i, 0:512], xkv_d.ap()[i][:, 0:512])
            for i in range(ct):
                nc.sync.dma_start(
                    xq_sb[:, i, 512:1024], xq_d.ap()[i][:, 512:1024])
            for chk in range(1, kch):
                for i in range(ct):
                    nc.sync.dma_start(
                        xkv_sb[:, i, chk * 512:(chk + 1) * 512],
                        xkv_d.ap()[i][:, chk * 512:(chk + 1) * 512])
            for i in range(ct):
                nc.sync.dma_start(wv_sb[:, i, :], wv_d.ap()[i])
            for chk in range(2, qch):
                for i in range(ct):
                    nc.sync.dma_start(
                        xq_sb[:, i, chk * 512:(chk + 1) * 512],
                        xq_d.ap()[i][:, chk * 512:(chk + 1) * 512])

        def q_chunk(chk):
            # q is double-fp8 (hi + lo residual): the query-side
            # quantization error dominated the attention error (it perturbs
            # each query row's logits incoherently, while key-side errors
            # mostly cancel via the denominator)
            ps = proj_pool.tile([128, 2, 512], F32, tag="pj", name="pj_t")
            sl = slice(chk * 512, (chk + 1) * 512)
            for o in range(ot):
                for c in range(ct):
                    nc.tensor.matmul(
                        ps[:, o, :], wq_sb[:, c, o * 128:(o + 1) * 128],
                        xq_sb[:, c, sl],
                        start=(c == 0), stop=(c == ct - 1))
            nc.vector.tensor_copy(q8h[:, :, sl], ps[:])
            nc.vector.tensor_tensor(
                q8l[:, :, sl], ps[:], q8h[:, :, sl],
                mybir.AluOpType.subtract)

        def k_chunk(rf, chk):
            xkv_sb = xkvs[rf % 2]
            k8 = k8s[rf % 2]
            ps = proj_pool.tile([128, 2, 512], F32, tag="pj", name="pj_t")
            for o in range(ot):
                for c in range(ct):
                    nc.tensor.matmul(
                        ps[:, o, :], wk_sb[:, c, o * 128:(o + 1) * 128],
                        xkv_sb[:, c, chk * 512:(chk + 1) * 512],
                        start=(c == 0), stop=(c == ct - 1))
            nc.vector.tensor_copy(
                k8[:, :, chk * 512:(chk + 1) * 512], ps[:])

        def s_exp(P8, k8, mj, n0):
            # one DoubleRow matmul pair (q hi + lo) per m-tile: full
            # 256-deep contraction per instruction; exp -> fp32 on ACT (the
            # bottleneck engine: keep it exp-only). The -1 -> fp8 P' sub
            # runs on DVE (2x_2p: all-SBUF operands) with a slice routed to
            # the otherwise-idle GPSIMD to keep DVE under ACT's pace.
            s_ps = s_pool.tile([128, 2, nblk], F32, tag="s", name="s_t")
            for half in range(2):
                mi = 2 * mj + half
                nc.tensor.matmul(
                    s_ps[:, half, :], k8[:, :, mi * 128:(mi + 1) * 128],
                    q8h[:, :, n0:n0 + nblk],
                    start=True, stop=False, perf_mode=DR)
                nc.tensor.matmul(
                    s_ps[:, half, :], k8[:, :, mi * 128:(mi + 1) * 128],
                    q8l[:, :, n0:n0 + nblk],
                    start=False, stop=True, perf_mode=DR)
            pf = pf_pool.tile([128, 2, nblk], F32, tag="pf", name="pf_t")
            nc.scalar.activation(pf[:], s_ps[:], EXP, scale=1.0 / TEMP)
            eng = nc.gpsimd if mj % 16 < subs_on_pool else nc.vector
            eng.tensor_scalar_add(
                P8[:, 2 * mj:2 * mj + 2, :], pf[:], -1.0)

        def pv(prev, blk, last=False):
            # per 128-query tile: pure fp8 DoubleRow accumulation of
            # P'^T v pairs over m (no dtype switches on PE); the
            # ones-row/colsum constant is restored on host
            P_tiles, v8 = prev
            P8 = P_tiles[blk]
            for ni in range(nt):
                y_ps = y_pool.tile([128, OUT + 1], F32, tag="y", name="y_t")
                for mj in range(mt // 2):
                    nc.tensor.matmul(
                        y_ps[:],
                        P8[:, 2 * mj:2 * mj + 2, ni * 128:(ni + 1) * 128],
                        v8[:, 2 * mj:2 * mj + 2, :],
                        start=(mj == 0), stop=(mj == mt // 2 - 1),
                        perf_mode=DR)
                y_sb = ysb_pool.tile([128, OUT + 1], BF16, tag="ysb",
                                     name="ysb_t")
                # the final drain's copy-outs ride on ACT so the DVE
                # stream doesn't end the program
                if last:
                    nc.scalar.copy(y_sb[:], y_ps[:])
                else:
                    nc.vector.tensor_copy(y_sb[:], y_ps[:])
                # y goes out on the otherwise-idle Pool queue so output
                # dispatch never contends with the SP input-DMA stream
                nc.gpsimd.dma_start(y_d.ap()[blk * nt + ni], y_sb[:])

        prev = None
        for rf in range(repeat_full):
            k8 = k8s[rf % 2]
            v8 = v8s[rf % 2]
            xkv_sb = xkvs[rf % 2]
            P_tiles = [None] * nb

            def new_P8(blk):
                P8 = p_pool.tile([128, mt, nblk], FP8, tag="p", name="P8")
                P_tiles[blk] = P8
                return P8

            if rf == 0:
                # cold prologue; for rf > 0 all of this was emitted inside
                # iteration rf-1's block-3 stream
                emit_dmas(0)
                q_chunk(0)
                q_chunk(1)
                for chk in range(kch):
                    k_chunk(0, chk)
            P8 = new_P8(0)
            for mj in range(mt // 2):
                s_exp(P8, k8, mj, 0)
            if prev is not None:
                pv(prev, 0)
            P8 = new_P8(1)
            for mj in range(mt // 2):
                s_exp(P8, k8, mj, nblk)
            if prev is not None:
                pv(prev, 1)
            for chk in range(2, qch):
                q_chunk(chk)
            P8 = new_P8(2)
            for mj in range(mt // 2):
                s_exp(P8, k8, mj, 2 * nblk)
            if prev is not None:
                pv(prev, 2)
            for mj in range(mt // 2):
                ps = proj_pool.tile([128, 2, 512], F32, tag="pj", name="pj_t")
                for half in range(2):
                    mi = 2 * mj + half
                    for c in range(ct):
                        nc.tensor.matmul(
                            ps[:, half, 0:OUT],
                            xkv_sb[:, c, mi * 128:(mi + 1) * 128],
                            wv_sb[:, c, :],
                            start=(c == 0), stop=(c == ct - 1))
                nc.vector.tensor_copy(
                    v8[:, 2 * mj:2 * mj + 2, 0:OUT], ps[:, :, 0:OUT])
            # next iteration's head: DMAs, q chunks 0/1, k chunk 0 up
            # front, the remaining k chunks interleaved into this
            # iteration's block-3 stream (all on the other xkv/k8 buffers)
            if rf + 1 < repeat_full:
                emit_dmas(rf + 1)
                q_chunk(0)
                q_chunk(1)
                k_chunk(rf + 1, 0)
            P8 = new_P8(3)
            for mj in range(mt // 2):
                s_exp(P8, k8, mj, 3 * nblk)
                if rf + 1 < repeat_full and mj % 2 == 1 and mj // 2 + 1 < kch:
                    k_chunk(rf + 1, mj // 2 + 1)
            if prev is not None:
                pv(prev, 3)
            prev = (P_tiles, v8)

        # final drain: the last iteration's PV stages
        for blk in range(nb):
            pv(prev, blk, last=(blk == nb - 1))
    nc.compile()
    return nc


def make_in_maps(x, xx, Wq, Wk, Wv, bc=2048, m=4096):
    """Host-side prep: slice/cast per-core inputs. Returns list of 8 dicts."""
    ct = C // 128
    wq_t = np.ascontiguousarray(Wq.T.astype(BFNP).reshape(ct, 128, OUT))
    wk_t = np.ascontiguousarray(Wk.T.astype(BFNP).reshape(ct, 128, OUT))
    wv_t = np.ascontiguousarray(Wv.T.astype(BFNP).reshape(ct, 128, OUT))
    halves = NCORES // B
    in_maps = []
    for core in range(NCORES):
        b, h = divmod(core, halves)
        xq = np.ascontiguousarray(
            x[b, :, h * bc:(h + 1) * bc].astype(BFNP).reshape(ct, 128, bc))
        xkv = np.ascontiguousarray(
            xx[b, :, :m].astype(BFNP).reshape(ct, 128, m))
        in_maps.append({"xq": xq, "xkv": xkv, "wqT": wq_t, "wkT": wk_t,
                        "wvT": wv_t})
    return in_maps


def gather_output(results, csums, bc=2048, m=4096):
    """Reassemble per-core (bc/128, 128, OUT+1) bf16 deviation tiles into
    (B, OUT, NSEQ): y = (dev[:, :OUT] + colsum_v) / (dev[:, OUT] + m)."""
    y = np.empty((B, OUT, NSEQ), dtype=np.float32)
    halves = NCORES // B
    for core, res in enumerate(results):
        b, h = divmod(core, halves)
        dev = res["y"].astype(np.float64).reshape(bc, OUT + 1)
        num = dev[:, :OUT] + csums[b][None, :]
        den = dev[:, OUT:] + float(m)
        y[b, :, h * bc:(h + 1) * bc] = (num / den).T.astype(np.float32)
    return y


_NC_CACHE = {}


def kernel(x, xx, Wq, Wk, Wv):
    x = np.asarray(x)
    xx = np.asarray(xx)
    Wq, Wk, Wv = np.asarray(Wq), np.asarray(Wk), np.asarray(Wv)
    key = "full"
    if key not in _NC_CACHE:
        _NC_CACHE[key] = build()
    nc = _NC_CACHE[key]
    in_maps = make_in_maps(x, xx, Wq, Wk, Wv)
    # exact ones-row constant per batch: colsum(v) = Wv @ sum_n xx[b,:,n]
    csums = [Wv.astype(np.float64) @ xx[b].astype(np.float64).sum(1)
             for b in range(B)]
    try:
        res = run_bass_kernel_spmd(nc, in_maps, core_ids=list(range(NCORES)))
    except Exception:
        # transient device state (e.g. a previous process left a core
        # unrecoverable) usually clears on retry
        res = run_bass_kernel_spmd(nc, in_maps, core_ids=list(range(NCORES)))
    return gather_output(res.results, csums)
